# revision 28
# baseline (speedup 1.0000x reference)
"""BiMamba Trainium2 kernel v4 (8 NeuronCores, SPMD).

Sharding: core = dir(2) x batch(2) x d_inner-half(2).

v4 (from v3 lessons; baseline v2 = 1295us):
- Scans are NOT packed (scan cost is ~2.3ns/col regardless of op size);
  per-(n,b) scans [128,1024] with carry via the scan `initial` operand.
- d1 = bsc*B and ch = h*C remain 4-state-packed DVE ops ([128,4096],
  0.56ns/col with stride-0 broadcast APs) - measured clean in v3.
- No GpSimd: its semaphore ops cost ~5.6us each (Q7 software polling).
- Depthwise conv: DVE (ts+3tt) for chunks 0-1 (DVE idle during A head),
  PE diagonal matmuls for chunks 2-3 (DVE busy with phase B h0).
- dt softplus: 8 Exps then 8 Lns batched (activation-table churn).
- apsum bufs=3 so PE inproj(m+1) does not wait on ACT xi copy of m.
"""

import sys

sys.path.insert(0, "/opt/trn_rl_repo")

import numpy as np
import ml_dtypes

import concourse.bass as bass
import concourse.bacc as bacc
import concourse.mybir as mybir
import concourse.tile as tile
from concourse import bass_utils

F32 = mybir.dt.float32
BF16 = mybir.dt.bfloat16
AF = mybir.ActivationFunctionType
ALU = mybir.AluOpType

B, L, DM = 2, 2048, 1024
DI = 2048
DH = DI // 2
N = 16
R = 64
K4 = 4
TC = 512
NCHUNK = L // TC          # 4
HALF = 1024
NH = L // HALF            # 2
NQ = 4                    # n-states per packed d1/ch op
NBLK_DM = DM // 128       # 8
NBLK_DH = DH // 128       # 8
NBLK_DF = DI // 128       # 16

_CACHED = {}


def _build_module():
    nc = bacc.Bacc("TRN2", target_bir_lowering=False, debug=False, num_devices=8)

    def din(name, shape, dt):
        return nc.dram_tensor(name, list(shape), dt, kind="ExternalInput").ap()

    xT = din("xT", (DM, L), BF16)
    w_in = din("w_in", (DM, DI + DH), BF16)
    w_xp = din("w_xp", (DI, 2 * N + R), BF16)
    w_dt = din("w_dt", (R, DH), BF16)
    w_out = din("w_out", (DH, DM), BF16)
    w_cdiag = din("w_cdiag", (128, NBLK_DF * K4 * 128), BF16)
    conv_w = din("conv_w", (DI, K4), F32)
    conv_b = din("conv_b", (DI, 1), F32)
    dt_b = din("dt_b", (DH, 1), F32)
    eye = din("eye", (128, 128), BF16)
    w_diag = din("w_diag", (128, NBLK_DH * 128), BF16)   # 8 diag(D) blocks
    out_d = nc.dram_tensor("out", [DM, L], BF16, kind="ExternalOutput").ap()
    z_spill = nc.dram_tensor("z_spill", [DH, L], BF16, kind="Internal").ap()
    xc_spill = nc.dram_tensor("xc_spill", [DH, L], BF16, kind="Internal").ap()
    dt_spill = nc.dram_tensor("dt_spill", [DH, L], BF16, kind="Internal").ap()
    bsc_spill = nc.dram_tensor("bsc_spill", [DH, L], BF16, kind="Internal").ap()
    bc_spill = nc.dram_tensor("bc_spill", [2 * N, L], BF16, kind="Internal").ap()

    with tile.TileContext(nc) as tc:
        _emit(nc, tc, xT, w_in, w_xp, w_dt, w_out, w_cdiag, conv_w, conv_b,
              dt_b, eye, w_diag, out_d, z_spill, xc_spill, dt_spill,
              bsc_spill, bc_spill)
    nc.compile()
    return nc


def _emit(nc, tc, xT, w_in, w_xp, w_dt, w_out, w_cdiag, conv_w, conv_b,
          dt_b, eye, w_diag, out_d, z_spill, xc_spill, dt_spill, bsc_spill,
          bc_spill):
    from contextlib import ExitStack
    ctx = ExitStack()
    with ctx:
        # ---------------- persistent weights/consts ----------------
        wpool = ctx.enter_context(tc.tile_pool(name="weights", bufs=1))
        conv_w_sb = wpool.tile([128, K4 * NBLK_DF], F32, tag="conv_w",
                               name="conv_w")
        nc.sync.dma_start(conv_w_sb[:], conv_w.rearrange("(k p) c -> p k c",
                                                         p=128))
        conv_b_sb = wpool.tile([128, NBLK_DF], F32, tag="conv_b", name="conv_b")
        nc.sync.dma_start(conv_b_sb[:], conv_b.rearrange("(k p) c -> p k c", p=128))
        dt_b_sb = wpool.tile([128, NBLK_DH], F32, tag="dt_b", name="dt_b")
        nc.sync.dma_start(dt_b_sb[:], dt_b.rearrange("(k p) c -> p k c", p=128))
        eye_sb = wpool.tile([128, 128], BF16, tag="eye", name="eye")
        nc.sync.dma_start(eye_sb[:], eye[:, :])
        w_diag_sb = wpool.tile([128, NBLK_DH * 128], BF16, tag="w_diag",
                               name="w_diag")
        nc.sync.dma_start(w_diag_sb[:], w_diag[:, :])
        w_xp_sb = []
        for k in range(NBLK_DF):
            t = wpool.tile([128, 2 * N + R], BF16, tag=f"w_xp{k}", name=f"w_xp{k}")
            nc.sync.dma_start(t[:], w_xp[k * 128:(k + 1) * 128, :])
            w_xp_sb.append(t)
        w_dt_sb = wpool.tile([R, DH], BF16, tag="w_dt", name="w_dt")
        nc.sync.dma_start(w_dt_sb[:], w_dt[:, :])

        # ---------------- resident phase-B state ----------------
        rpool = ctx.enter_context(tc.tile_pool(name="resident", bufs=1))
        carry = rpool.tile([128, 128], BF16, tag="carry", name="carry")
        spool = ctx.enter_context(tc.tile_pool(name="sgate", bufs=2))
        s_ref = [[None] * NBLK_DH for _ in range(NH)]

        apsum = ctx.enter_context(
            tc.tile_pool(name="phaseA_ps", bufs=3, space="PSUM"))
        apsum1 = ctx.enter_context(
            tc.tile_pool(name="phaseA_ps1", bufs=1, space="PSUM"))
        bpsum = ctx.enter_context(
            tc.tile_pool(name="phaseB_ps", bufs=1, space="PSUM"))
        bcpool = ctx.enter_context(tc.tile_pool(name="bcast", bufs=2))
        bpool = ctx.enter_context(tc.tile_pool(name="phaseB", bufs=2))
        qpool = ctx.enter_context(tc.tile_pool(name="quads", bufs=2))
        qpool1 = ctx.enter_context(tc.tile_pool(name="quads1", bufs=1))

        def make_phaseB_steps(t0, tl, carry_in, carry_out):
            """Phase-B steps for time segment [t0, t0+tl).

            Steps per b-pair: head, then per quad q: [bc-load, b0, b1], tail.
            Carries chain segments via the scan `initial` operand.
            """
            h = t0 // HALF
            steps = []
            for p in range(NBLK_DH // 2):
                bs = [2 * p, 2 * p + 1]
                st = {}

                def pair_head(p=p, bs=bs, st=st):
                    st["y2"] = {}
                    st["dt"] = {}
                    st["bsc"] = {}
                    for i, b in enumerate(bs):
                        st["y2"][b] = bpsum.tile([128, tl], F32,
                                                 tag=f"y2_{i}", name=f"y2_{b}")
                        dtr = bpool.tile([128, tl], BF16, tag=f"dtr{i}",
                                         name=f"dtr{b}")
                        nc.sync.dma_start(
                            dtr[:], dt_spill[b * 128:(b + 1) * 128, t0:t0 + tl])
                        st["dt"][b] = dtr
                        bsr = bpool.tile([128, tl], BF16, tag=f"bsr{i}",
                                         name=f"bsr{b}")
                        nc.sync.dma_start(
                            bsr[:], bsc_spill[b * 128:(b + 1) * 128, t0:t0 + tl])
                        st["bsc"][b] = bsr
                steps.append(pair_head)

                for q in range(N // NQ):
                    def q_head(q=q, st=st):
                        BQ = bcpool.tile([128, NQ * tl], BF16, tag="BQ",
                                         name="BQ")
                        CQ = bcpool.tile([128, NQ * tl], BF16, tag="CQ",
                                         name="CQ")
                        for s in range(NQ):
                            n = q * NQ + s
                            nc.sync.dma_start(
                                BQ[:, s * tl:(s + 1) * tl],
                                bc_spill[n:n + 1, t0:t0 + tl]
                                .partition_broadcast(128))
                            nc.sync.dma_start(
                                CQ[:, s * tl:(s + 1) * tl],
                                bc_spill[N + n:N + n + 1, t0:t0 + tl]
                                .partition_broadcast(128))
                        st["BQ"], st["CQ"] = BQ, CQ
                    steps.append(q_head)

                    for b in bs:
                        def nb_step(q=q, b=b, st=st):
                            dA4 = qpool.tile([128, NQ * tl], BF16, tag="dA4",
                                             name="dA4")
                            for s in range(NQ):
                                n = q * NQ + s
                                nc.scalar.activation(
                                    dA4[:, s * tl:(s + 1) * tl],
                                    st["dt"][b][:], AF.Exp, scale=-float(n + 1))
                            d14 = qpool.tile([128, NQ * tl], BF16, tag="d14",
                                             name="d14")
                            nc.vector.tensor_tensor(
                                d14[:].rearrange("p (s t) -> p s t", s=NQ),
                                st["bsc"][b][:].unsqueeze(1)
                                .broadcast_to([128, NQ, tl]),
                                st["BQ"][:].rearrange("p (s t) -> p s t", s=NQ),
                                ALU.mult)
                            hs4 = qpool.tile([128, NQ * tl], BF16, tag="hs4",
                                             name="hs4")
                            for s in range(NQ):
                                n = q * NQ + s
                                idx = b * N + n
                                off = s * tl
                                init = (carry[:, idx:idx + 1] if carry_in
                                        else 0.0)
                                nc.vector.tensor_tensor_scan(
                                    hs4[:, off:off + tl],
                                    dA4[:, off:off + tl],
                                    d14[:, off:off + tl],
                                    init, ALU.mult, ALU.add)
                                if carry_out:
                                    nc.scalar.copy(
                                        carry[:, idx:idx + 1],
                                        hs4[:, off + tl - 1:off + tl])
                            ch4 = qpool1.tile([128, NQ * tl], BF16, tag="ch4",
                                              name="ch4")
                            nc.vector.tensor_tensor(ch4[:], hs4[:],
                                                    st["CQ"][:], ALU.mult)
                            for s in range(NQ):
                                for u in range(tl // TC):
                                    nc.tensor.matmul(
                                        st["y2"][b][:, u * TC:(u + 1) * TC],
                                        eye_sb[:],
                                        ch4[:, s * tl + u * TC:
                                            s * tl + (u + 1) * TC],
                                        start=(q == 0 and s == 0), stop=False,
                                        skip_group_check=True)
                        steps.append(nb_step)

                def pair_tail(p=p, bs=bs, st=st):
                    for b in bs:
                        xcr = bpool.tile([128, tl], BF16, tag="xcr",
                                         name="xcr")
                        nc.sync.dma_start(
                            xcr[:],
                            xc_spill[b * 128:(b + 1) * 128, t0:t0 + tl])
                        for u in range(tl // TC):
                            nc.tensor.matmul(
                                st["y2"][b][:, u * TC:(u + 1) * TC],
                                w_diag_sb[:, b * 128:(b + 1) * 128],
                                xcr[:, u * TC:(u + 1) * TC],
                                start=False, stop=True, skip_group_check=True)
                        zs = bpool.tile([128, tl], BF16, tag="zs", name="zs")
                        nc.sync.dma_start(
                            zs[:], z_spill[b * 128:(b + 1) * 128, t0:t0 + tl])
                        if t0 % HALF == 0:
                            s_ref[h][b] = spool.tile([128, HALF], BF16,
                                                     tag=f"s{b}",
                                                     name=f"s{h}_{b}")
                        o0 = t0 % HALF
                        nc.vector.tensor_tensor(
                            s_ref[h][b][:, o0:o0 + tl], st["y2"][b][:], zs[:],
                            ALU.mult)
                steps.append(pair_tail)
            return steps

        def make_phaseC_steps(h, w_out_sb):
            t0 = h * HALF
            steps = []
            for m in range(NBLK_DM):
                for u in range(HALF // TC):
                    def c_step(m=m, u=u):
                        qt = t0 + u * TC
                        # reuse phase-A inproj PSUM (phase A is done by now)
                        ps = apsum.tile([128, TC], F32, tag="inproj",
                                        name="oproj")
                        for k in range(NBLK_DH):
                            nc.tensor.matmul(
                                ps[:], w_out_sb[k][:, m * 128:(m + 1) * 128],
                                s_ref[h][k][:, u * TC:(u + 1) * TC],
                                start=(k == 0), stop=(k == NBLK_DH - 1))
                        ot = bpool.tile([128, TC], BF16, tag="ot", name="ot")
                        nc.scalar.activation(ot[:], ps[:], AF.Copy)
                        nc.sync.dma_start(
                            out_d[m * 128:(m + 1) * 128, qt:qt + TC], ot[:])
                    steps.append(c_step)
            return steps

        # ================= Phase A (+ interleaved B h0) =================
        # h0 is processed in two 512-quarters so the first quarter (which
        # only needs chunk-0 outputs) can start while chunk 1 is on the PE.
        stepsB0 = iter(list(make_phaseB_steps(0, TC, False, True))
                       + list(make_phaseB_steps(TC, TC, True, True)))
        with tc.tile_pool(name="phaseA_w", bufs=3) as wpa, \
             tc.tile_pool(name="phaseA", bufs=2) as apool, \
             tc.tile_pool(name="phaseA_dt", bufs=1) as dtpool, \
             tc.tile_pool(name="phaseA_x", bufs=1) as xpool, \
             tc.tile_pool(name="phaseA_xc", bufs=1) as xcpool, \
             tc.tile_pool(name="phaseA_misc", bufs=1) as mpool:
            halo = [mpool.tile([128, 3], BF16, tag=f"halo{b}", name=f"halo{b}")
                    for b in range(NBLK_DF)]
            for b in range(NBLK_DF):
                nc.vector.memset(halo[b][:], 0.0)
            w_in_r = w_in.rearrange("(k p) c -> p k c", p=128)  # [128,8,3072]
            for c in range(NCHUNK):
                t0 = c * TC
                conv_on_pe = c >= 1  # chunk 0: DVE is idle, keep conv there
                x_sb = []
                for k in range(NBLK_DM):
                    t = xpool.tile([128, TC], BF16, tag=f"x{k}", name=f"x{k}")
                    nc.sync.dma_start(t[:], xT[k * 128:(k + 1) * 128, t0:t0 + TC])
                    x_sb.append(t)
                xc_chunk = []
                for m in range(NBLK_DF + NBLK_DH):
                    wm = wpa.tile([128, (NBLK_DM + K4) * 128], BF16,
                                  tag="w_in_m", name=f"w_in_m{m}")
                    nc.sync.dma_start(
                        wm[:, 0:NBLK_DM * 128]
                        .rearrange("p (k c) -> p k c", k=NBLK_DM),
                        w_in_r[:, :, m * 128:(m + 1) * 128])
                    if conv_on_pe and m < NBLK_DF:
                        nc.sync.dma_start(
                            wm[:, NBLK_DM * 128:],
                            w_cdiag[:, m * K4 * 128:(m + 1) * K4 * 128])
                    ps = apsum.tile([128, TC], F32, tag="inproj", name="inproj")
                    for k in range(NBLK_DM):
                        nc.tensor.matmul(ps[:], wm[:, k * 128:(k + 1) * 128],
                                         x_sb[k][:], start=(k == 0),
                                         stop=(k == NBLK_DM - 1))
                    if m < NBLK_DF:
                        xi = apool.tile([128, 3 + TC], BF16, tag="xi", name="xi")
                        nc.gpsimd.tensor_copy(xi[:, 0:3], halo[m][:])
                        nc.scalar.activation(xi[:, 3:3 + TC], ps[:], AF.Copy)
                        nc.scalar.activation(halo[m][:], ps[:, TC - 3:TC],
                                             AF.Copy)
                        xc_t = xcpool.tile([128, TC], BF16, tag=f"xco{m}",
                                           name=f"xco{m}")
                        if conv_on_pe:
                            psc = apsum.tile([128, TC], F32, tag="inproj",
                                             name="convps")
                            for kk in range(K4):
                                nc.tensor.matmul(
                                    psc[:],
                                    wm[:, (NBLK_DM + kk) * 128:
                                       (NBLK_DM + kk + 1) * 128],
                                    xi[:, kk:kk + TC],
                                    start=(kk == 0), stop=(kk == K4 - 1))
                            nc.scalar.activation(xc_t[:], psc[:], AF.Silu,
                                                 bias=conv_b_sb[:, m:m + 1])
                        else:
                            acc = apool.tile([128, TC], BF16, tag="convacc",
                                             name="convacc")
                            tmp = apool.tile([128, TC], BF16, tag="convtmp",
                                             name="convtmp")
                            nc.vector.tensor_scalar(
                                acc[:], xi[:, 0:TC],
                                conv_w_sb[:, m * K4:m * K4 + 1], None, ALU.mult)
                            for kk in range(1, K4):
                                nc.vector.tensor_scalar(
                                    tmp[:], xi[:, kk:kk + TC],
                                    conv_w_sb[:, m * K4 + kk:m * K4 + kk + 1],
                                    None, ALU.mult)
                                nc.vector.tensor_tensor(acc[:], acc[:], tmp[:],
                                                        ALU.add)
                            nc.scalar.activation(xc_t[:], acc[:], AF.Silu,
                                                 bias=conv_b_sb[:, m:m + 1])
                        if m < NBLK_DH:
                            nc.sync.dma_start(
                                xc_spill[m * 128:(m + 1) * 128, t0:t0 + TC],
                                xc_t[:])
                        xc_chunk.append(xc_t)
                    else:
                        zb = m - NBLK_DF
                        zt = apool.tile([128, TC], BF16, tag="zt", name="zt")
                        nc.scalar.activation(zt[:], ps[:], AF.Silu)
                        nc.sync.dma_start(
                            z_spill[zb * 128:(zb + 1) * 128, t0:t0 + TC], zt[:])

                # xproj
                ps96 = apsum1.tile([R + 2 * N, TC], F32, tag="xpdt", name="xproj")
                for k in range(NBLK_DF):
                    nc.tensor.matmul(ps96[:], w_xp_sb[k][:], xc_chunk[k][:],
                                     start=(k == 0), stop=(k == NBLK_DF - 1))
                xdbl = apool.tile([R + 2 * N, TC], BF16, tag="xdbl", name="xdbl")
                nc.scalar.activation(xdbl[:], ps96[:], AF.Copy)
                nc.sync.dma_start(bc_spill[:, t0:t0 + TC], xdbl[R:R + 2 * N, :])
                # dt proj + softplus; Exp x4 then Ln x4 batched so the
                # activation-table pass emits one load per run, not per op.
                spes = []
                for mb in range(NBLK_DH):
                    psd = apsum1.tile([128, TC], F32, tag="xpdt", name="dtproj")
                    nc.tensor.matmul(psd[:], w_dt_sb[:, mb * 128:(mb + 1) * 128],
                                     xdbl[0:R, :], start=True, stop=True)
                    spe = dtpool.tile([128, TC], BF16, tag=f"spe{mb % 4}",
                                      name="spe")
                    nc.scalar.activation(spe[:], psd[:], AF.Exp,
                                         bias=dt_b_sb[:, mb:mb + 1])
                    spes.append(spe)
                    if mb % 4 == 3:
                        for j, sp in enumerate(spes):
                            mbj = mb - 3 + j
                            dtt = dtpool.tile([128, TC], BF16, tag=f"dtt{j % 2}",
                                              name="dtt")
                            nc.scalar.activation(dtt[:], sp[:], AF.Ln, bias=1.0)
                            nc.sync.dma_start(
                                dt_spill[mbj * 128:(mbj + 1) * 128, t0:t0 + TC],
                                dtt[:])
                            bst = dtpool.tile([128, TC], BF16, tag=f"bst{j % 2}",
                                              name="bst")
                            nc.gpsimd.tensor_tensor(bst[:], dtt[:],
                                                    xc_chunk[mbj][:], ALU.mult)
                            nc.sync.dma_start(
                                bsc_spill[mbj * 128:(mbj + 1) * 128,
                                          t0:t0 + TC], bst[:])
                        spes = []
                # inject phase-B h0 quarter-pairs as their inputs appear:
                # after chunk 0, quarter Q0 (t<512) is runnable; 28 steps
                # per chunk keeps DVE fed through chunks 1-3.
                for _ in range(28):
                    nxt = next(stepsB0, None)
                    if nxt is not None:
                        nxt()

        # ====== drain B h0; load w_out; B h1 interleaved with C h0 ======
        for nxt in stepsB0:
            nxt()
        with tc.tile_pool(name="phaseC_w", bufs=1) as wpc:
            w_out_sb = []
            for k in range(NBLK_DH):
                t = wpc.tile([128, DM], BF16, tag=f"w_out{k}", name=f"w_out{k}")
                nc.sync.dma_start(t[:], w_out[k * 128:(k + 1) * 128, :])
                w_out_sb.append(t)
            stepsC0 = iter(make_phaseC_steps(0, w_out_sb))
            stepsB1 = make_phaseB_steps(HALF, HALF, True, False)
            for i, stp in enumerate(stepsB1):
                stp()
                if i % 14 >= 10:
                    nxt = next(stepsC0, None)
                    if nxt is not None:
                        nxt()
            for nxt in stepsC0:
                nxt()
            for stp in make_phaseC_steps(1, w_out_sb):
                stp()


def _prep_inputs(inputs):
    """Build the 8 per-core input maps from full inputs (numpy fp32)."""
    bf = ml_dtypes.bfloat16
    x = np.asarray(inputs["x"], np.float32)
    maps = []
    for core in range(8):
        dire, bat, half = core // 4, (core // 2) % 2, core % 2
        p = "fwd" if dire == 0 else "bwd"
        in_W = np.asarray(inputs[p + "_in_W"], np.float32)
        conv_w = np.asarray(inputs[p + "_conv_w"], np.float32)
        conv_b = np.asarray(inputs[p + "_conv_b"], np.float32)
        xproj_W = np.asarray(inputs[p + "_xproj_W"], np.float32)
        dt_W = np.asarray(inputs[p + "_dt_W"], np.float32)
        dt_b = np.asarray(inputs[p + "_dt_b"], np.float32)
        A_log = np.asarray(inputs[p + "_A_log"], np.float32)
        Dvec = np.asarray(inputs[p + "_D"], np.float32)
        out_W = np.asarray(inputs[p + "_out_W"], np.float32)
        proj_W = np.asarray(inputs["proj_W"], np.float32)

        # the kernel generates dA = exp(-n*dt); verify A has that structure
        A = -np.exp(A_log)
        assert np.allclose(A, -np.arange(1, N + 1, dtype=np.float32)[None, :]
                           .repeat(DI, 0), atol=1e-4), "unexpected A structure"

        own = slice(half * DH, (half + 1) * DH)
        xb = x[bat]
        if dire == 1:
            xb = xb[::-1]
        perm = np.concatenate([np.arange(half * DH, (half + 1) * DH),
                               np.arange((1 - half) * DH, (2 - half) * DH)])
        w_in_cat = np.concatenate(
            [in_W[perm], in_W[DI + half * DH:DI + (half + 1) * DH]], 0)
        W_eff = proj_W[:, dire * DM:(dire + 1) * DM] @ out_W   # (DM, DI)
        D_own = Dvec[own]
        w_diag = np.zeros((128, NBLK_DH * 128), np.float32)
        for b in range(NBLK_DH):
            w_diag[:, b * 128:(b + 1) * 128] = np.diag(D_own[b * 128:(b + 1) * 128])
        cw = conv_w[perm]    # (DI, 4)
        w_cdiag = np.zeros((128, NBLK_DF * K4 * 128), np.float32)
        for m in range(NBLK_DF):
            for kk in range(K4):
                j = (m * K4 + kk) * 128
                w_cdiag[:, j:j + 128] = np.diag(cw[m * 128:(m + 1) * 128, kk])
        m = {
            "xT": np.ascontiguousarray(xb.T).astype(bf),
            "w_in": np.ascontiguousarray(w_in_cat.T).astype(bf),
            "w_xp": np.ascontiguousarray(xproj_W[:, perm].T).astype(bf),
            "w_dt": np.ascontiguousarray(dt_W[own].T).astype(bf),
            "w_out": np.ascontiguousarray(W_eff[:, own].T).astype(bf),
            "w_cdiag": np.ascontiguousarray(w_cdiag).astype(bf),
            "conv_w": np.ascontiguousarray(cw),
            "conv_b": np.ascontiguousarray(conv_b[perm][:, None]),
            "dt_b": np.ascontiguousarray(dt_b[own][:, None]),
            "eye": np.eye(128, dtype=np.float32).astype(bf),
            "w_diag": np.ascontiguousarray(w_diag).astype(bf),
        }
        maps.append(m)
    return maps


def _unshard(results, inputs):
    parts = [r["out"].astype(np.float32) for r in results]
    proj_b = np.asarray(inputs["proj_b"], np.float32)
    out = np.empty((B, L, DM), np.float32)
    for bat in range(2):
        fwd = parts[0 * 4 + bat * 2 + 0] + parts[0 * 4 + bat * 2 + 1]
        bwd = parts[1 * 4 + bat * 2 + 0] + parts[1 * 4 + bat * 2 + 1]
        out[bat] = (fwd + bwd[:, ::-1]).T + proj_b[None, :]
    return out


def kernel(**inputs):
    if "nc" not in _CACHED:
        _CACHED["nc"] = _build_module()
    nc = _CACHED["nc"]
    maps = _prep_inputs(inputs)
    res = bass_utils.run_bass_kernel_spmd(nc, maps, core_ids=list(range(8)))
    return _unshard(res.results, inputs)


# revision 35
# speedup vs baseline: 1.2387x; 1.2387x over previous
"""BiMamba Trainium2 kernel v4 (8 NeuronCores, SPMD).

Sharding: core = dir(2) x batch(2) x d_inner-half(2).

v4 (from v3 lessons; baseline v2 = 1295us):
- Scans are NOT packed (scan cost is ~2.3ns/col regardless of op size);
  per-(n,b) scans [128,1024] with carry via the scan `initial` operand.
- d1 = bsc*B and ch = h*C remain 4-state-packed DVE ops ([128,4096],
  0.56ns/col with stride-0 broadcast APs) - measured clean in v3.
- No GpSimd: its semaphore ops cost ~5.6us each (Q7 software polling).
- Depthwise conv: DVE (ts+3tt) for chunks 0-1 (DVE idle during A head),
  PE diagonal matmuls for chunks 2-3 (DVE busy with phase B h0).
- dt softplus: 8 Exps then 8 Lns batched (activation-table churn).
- apsum bufs=3 so PE inproj(m+1) does not wait on ACT xi copy of m.
"""

import sys

sys.path.insert(0, "/opt/trn_rl_repo")

import numpy as np
import ml_dtypes

import concourse.bass as bass
import concourse.bacc as bacc
import concourse.mybir as mybir
import concourse.tile as tile
from concourse import bass_utils

F32 = mybir.dt.float32
BF16 = mybir.dt.bfloat16
AF = mybir.ActivationFunctionType
ALU = mybir.AluOpType

B, L, DM = 2, 2048, 1024
DI = 2048
DH = DI // 2
N = 16
R = 64
K4 = 4
TC = 512
NCHUNK = L // TC          # 4
HALF = 1024
NH = L // HALF            # 2
NQ = 4                    # n-states per packed d1/ch op
NBLK_DM = DM // 128       # 8
NBLK_DH = DH // 128       # 8
NBLK_DF = DI // 128       # 16

_CACHED = {}


def _build_module():
    nc = bacc.Bacc("TRN2", target_bir_lowering=False, debug=False, num_devices=8)

    def din(name, shape, dt):
        return nc.dram_tensor(name, list(shape), dt, kind="ExternalInput").ap()

    xT = din("xT", (DM, L), BF16)
    w_in = din("w_in", (DM, DI + DH), BF16)
    w_xp = din("w_xp", (DI, 2 * N + R), BF16)
    w_dt = din("w_dt", (R, DH), BF16)
    w_out = din("w_out", (DH, DM), BF16)
    w_cdiag = din("w_cdiag", (128, NBLK_DF * K4 * 128), BF16)
    conv_w = din("conv_w", (DI, K4), F32)
    conv_b = din("conv_b", (DI, 1), F32)
    dt_b = din("dt_b", (DH, 1), F32)
    eye = din("eye", (128, 128), BF16)
    w_diag = din("w_diag", (128, NBLK_DH * 128), BF16)   # 8 diag(D) blocks
    out_d = nc.dram_tensor("out", [DM, L], BF16, kind="ExternalOutput").ap()
    z_spill = nc.dram_tensor("z_spill", [DH, L], BF16, kind="Internal").ap()
    xc_spill = nc.dram_tensor("xc_spill", [DH, L], BF16, kind="Internal").ap()
    dt_spill = nc.dram_tensor("dt_spill", [DH, L], BF16, kind="Internal").ap()
    bsc_spill = nc.dram_tensor("bsc_spill", [DH, L], BF16, kind="Internal").ap()
    bc_spill = nc.dram_tensor("bc_spill", [2 * N, L], BF16, kind="Internal").ap()

    with tile.TileContext(nc) as tc:
        _emit(nc, tc, xT, w_in, w_xp, w_dt, w_out, w_cdiag, conv_w, conv_b,
              dt_b, eye, w_diag, out_d, z_spill, xc_spill, dt_spill,
              bsc_spill, bc_spill)
    nc.compile()
    return nc


def _emit(nc, tc, xT, w_in, w_xp, w_dt, w_out, w_cdiag, conv_w, conv_b,
          dt_b, eye, w_diag, out_d, z_spill, xc_spill, dt_spill, bsc_spill,
          bc_spill):
    from contextlib import ExitStack
    ctx = ExitStack()
    with ctx:
        # ---------------- persistent weights/consts ----------------
        wpool = ctx.enter_context(tc.tile_pool(name="weights", bufs=1))
        conv_w_sb = wpool.tile([128, K4 * NBLK_DF], F32, tag="conv_w",
                               name="conv_w")
        nc.sync.dma_start(conv_w_sb[:], conv_w.rearrange("(k p) c -> p k c",
                                                         p=128))
        conv_b_sb = wpool.tile([128, NBLK_DF], F32, tag="conv_b", name="conv_b")
        nc.sync.dma_start(conv_b_sb[:], conv_b.rearrange("(k p) c -> p k c", p=128))
        dt_b_sb = wpool.tile([128, NBLK_DH], F32, tag="dt_b", name="dt_b")
        nc.sync.dma_start(dt_b_sb[:], dt_b.rearrange("(k p) c -> p k c", p=128))
        eye_sb = wpool.tile([128, 128], BF16, tag="eye", name="eye")
        nc.sync.dma_start(eye_sb[:], eye[:, :])
        w_diag_sb = wpool.tile([128, NBLK_DH * 128], BF16, tag="w_diag",
                               name="w_diag")
        nc.sync.dma_start(w_diag_sb[:], w_diag[:, :])
        w_xp_sb = []
        for k in range(NBLK_DF):
            t = wpool.tile([128, 2 * N + R], BF16, tag=f"w_xp{k}", name=f"w_xp{k}")
            nc.sync.dma_start(t[:], w_xp[k * 128:(k + 1) * 128, :])
            w_xp_sb.append(t)
        w_dt_sb = wpool.tile([R, DH], BF16, tag="w_dt", name="w_dt")
        nc.sync.dma_start(w_dt_sb[:], w_dt[:, :])

        # ---------------- resident phase-B state ----------------
        rpool = ctx.enter_context(tc.tile_pool(name="resident", bufs=1))
        carry = rpool.tile([128, 128], BF16, tag="carry", name="carry")
        spool = ctx.enter_context(tc.tile_pool(name="sgate", bufs=2))
        s_ref = [[None] * NBLK_DH for _ in range(NH)]

        apsum = ctx.enter_context(
            tc.tile_pool(name="phaseA_ps", bufs=3, space="PSUM"))
        apsum1 = ctx.enter_context(
            tc.tile_pool(name="phaseA_ps1", bufs=1, space="PSUM"))
        bpsum = ctx.enter_context(
            tc.tile_pool(name="phaseB_ps", bufs=1, space="PSUM"))
        bcpool = ctx.enter_context(tc.tile_pool(name="bcast", bufs=2))
        bpool = ctx.enter_context(tc.tile_pool(name="phaseB", bufs=2))
        qpool = ctx.enter_context(tc.tile_pool(name="quads", bufs=2))
        qpool1 = ctx.enter_context(tc.tile_pool(name="quads1", bufs=1))

        def make_phaseB_steps(h):
            """Steps per pair: head, then per q: [bc-load, b0, b1], tail."""
            t0 = h * HALF
            steps = []
            for p in range(NBLK_DH // 2):
                bs = [2 * p, 2 * p + 1]
                st = {}

                def pair_head(p=p, bs=bs, st=st):
                    st["y2"] = {}
                    st["dt"] = {}
                    st["bsc"] = {}
                    for i, b in enumerate(bs):
                        st["y2"][b] = bpsum.tile([128, HALF], F32,
                                                 tag=f"y2_{i}", name=f"y2_{b}")
                        dtr = bpool.tile([128, HALF], BF16, tag=f"dtr{i}",
                                         name=f"dtr{b}")
                        nc.gpsimd.dma_start(
                            dtr[:], dt_spill[b * 128:(b + 1) * 128, t0:t0 + HALF])
                        st["dt"][b] = dtr
                        bsr = bpool.tile([128, HALF], BF16, tag=f"bsr{i}",
                                         name=f"bsr{b}")
                        nc.gpsimd.dma_start(
                            bsr[:], bsc_spill[b * 128:(b + 1) * 128, t0:t0 + HALF])
                        st["bsc"][b] = bsr
                steps.append(pair_head)

                for q in range(N // NQ):
                    def q_head(q=q, st=st):
                        BQ = bcpool.tile([128, NQ * HALF], BF16, tag="BQ",
                                         name="BQ")
                        CQ = bcpool.tile([128, NQ * HALF], BF16, tag="CQ",
                                         name="CQ")
                        for s2 in range(NQ // 2):
                            n = q * NQ + 2 * s2
                            nc.sync.dma_start(
                                BQ[:].rearrange("p (s t) -> p s t", s=NQ)
                                [:, 2 * s2:2 * s2 + 2, :],
                                bc_spill[n:n + 2, t0:t0 + HALF]
                                .partition_broadcast(128))
                            nc.sync.dma_start(
                                CQ[:].rearrange("p (s t) -> p s t", s=NQ)
                                [:, 2 * s2:2 * s2 + 2, :],
                                bc_spill[N + n:N + n + 2, t0:t0 + HALF]
                                .partition_broadcast(128))
                        st["BQ"], st["CQ"] = BQ, CQ
                    steps.append(q_head)

                    for b in bs:
                        def nb_step(q=q, b=b, st=st):
                            dA4 = qpool.tile([128, NQ * HALF], BF16, tag="dA4",
                                             name="dA4")
                            for s in range(NQ):
                                n = q * NQ + s
                                nc.scalar.activation(
                                    dA4[:, s * HALF:(s + 1) * HALF],
                                    st["dt"][b][:], AF.Exp, scale=-float(n + 1))
                            d14 = qpool.tile([128, NQ * HALF], BF16, tag="d14",
                                             name="d14")
                            nc.vector.tensor_tensor(
                                d14[:].rearrange("p (s t) -> p s t", s=NQ),
                                st["bsc"][b][:].unsqueeze(1)
                                .broadcast_to([128, NQ, HALF]),
                                st["BQ"][:].rearrange("p (s t) -> p s t", s=NQ),
                                ALU.mult)
                            hs4 = qpool.tile([128, NQ * HALF], BF16, tag="hs4",
                                             name="hs4")
                            for s in range(NQ):
                                n = q * NQ + s
                                idx = b * N + n
                                off = s * HALF
                                init = (0.0 if h == 0
                                        else carry[:, idx:idx + 1])
                                nc.vector.tensor_tensor_scan(
                                    hs4[:, off:off + HALF],
                                    dA4[:, off:off + HALF],
                                    d14[:, off:off + HALF],
                                    init, ALU.mult, ALU.add)
                                if h + 1 < NH:
                                    nc.gpsimd.tensor_copy(
                                        carry[:, idx:idx + 1],
                                        hs4[:, off + HALF - 1:off + HALF])
                            ch4 = qpool1.tile([128, NQ * HALF], BF16, tag="ch4",
                                              name="ch4")
                            nc.vector.tensor_tensor(ch4[:], hs4[:],
                                                    st["CQ"][:], ALU.mult)
                            for s in range(NQ):
                                for u in range(HALF // TC):
                                    nc.tensor.matmul(
                                        st["y2"][b][:, u * TC:(u + 1) * TC],
                                        eye_sb[:],
                                        ch4[:, s * HALF + u * TC:
                                            s * HALF + (u + 1) * TC],
                                        start=(q == 0 and s == 0), stop=False,
                                        skip_group_check=True)
                        steps.append(nb_step)

                def pair_tail(p=p, bs=bs, st=st):
                    for b in bs:
                        xcr = bpool.tile([128, HALF], BF16, tag="xcr",
                                         name="xcr")
                        nc.gpsimd.dma_start(
                            xcr[:],
                            xc_spill[b * 128:(b + 1) * 128, t0:t0 + HALF])
                        for u in range(HALF // TC):
                            nc.tensor.matmul(
                                st["y2"][b][:, u * TC:(u + 1) * TC],
                                w_diag_sb[:, b * 128:(b + 1) * 128],
                                xcr[:, u * TC:(u + 1) * TC],
                                start=False, stop=True, skip_group_check=True)
                        zs = bpool.tile([128, HALF], BF16, tag="zs", name="zs")
                        nc.gpsimd.dma_start(
                            zs[:], z_spill[b * 128:(b + 1) * 128, t0:t0 + HALF])
                        s_t = spool.tile([128, HALF], BF16, tag=f"s{b}",
                                         name=f"s{h}_{b}")
                        nc.vector.tensor_tensor(s_t[:], st["y2"][b][:], zs[:],
                                                ALU.mult)
                        s_ref[h][b] = s_t
                steps.append(pair_tail)
            return steps

        def make_phaseC_steps(h, w_out_sb):
            t0 = h * HALF
            steps = []
            for m in range(NBLK_DM):
                for u in range(HALF // TC):
                    def c_step(m=m, u=u):
                        qt = t0 + u * TC
                        # reuse phase-A inproj PSUM (phase A is done by now)
                        ps = apsum.tile([128, TC], F32, tag="inproj",
                                        name="oproj")
                        for k in range(NBLK_DH):
                            nc.tensor.matmul(
                                ps[:], w_out_sb[k][:, m * 128:(m + 1) * 128],
                                s_ref[h][k][:, u * TC:(u + 1) * TC],
                                start=(k == 0), stop=(k == NBLK_DH - 1))
                        ot = bpool.tile([128, TC], BF16, tag="ot", name="ot")
                        nc.scalar.activation(ot[:], ps[:], AF.Copy)
                        nc.sync.dma_start(
                            out_d[m * 128:(m + 1) * 128, qt:qt + TC], ot[:])
                    steps.append(c_step)
            return steps

        # ================= Phase A (+ interleaved B h0) =================
        stepsB0 = iter(make_phaseB_steps(0))
        with tc.tile_pool(name="phaseA_w", bufs=3) as wpa, \
             tc.tile_pool(name="phaseA", bufs=2) as apool, \
             tc.tile_pool(name="phaseA_dt", bufs=1) as dtpool, \
             tc.tile_pool(name="phaseA_x", bufs=1) as xpool, \
             tc.tile_pool(name="phaseA_xc", bufs=1) as xcpool, \
             tc.tile_pool(name="phaseA_misc", bufs=1) as mpool:
            halo = [mpool.tile([128, 3], BF16, tag=f"halo{b}", name=f"halo{b}")
                    for b in range(NBLK_DF)]
            for b in range(NBLK_DF):
                nc.vector.memset(halo[b][:], 0.0)
            w_in_r = w_in.rearrange("(k p) c -> p k c", p=128)  # [128,8,3072]
            for c in range(NCHUNK):
                t0 = c * TC
                conv_on_pe = c >= 2  # chunks 0-1: DVE idle, keep conv there
                x_sb = []
                for k in range(NBLK_DM):
                    t = xpool.tile([128, TC], BF16, tag=f"x{k}", name=f"x{k}")
                    nc.sync.dma_start(t[:], xT[k * 128:(k + 1) * 128, t0:t0 + TC])
                    x_sb.append(t)
                xc_chunk = []
                for m in range(NBLK_DF + NBLK_DH):
                    wm = wpa.tile([128, (NBLK_DM + K4) * 128], BF16,
                                  tag="w_in_m", name=f"w_in_m{m}")
                    nc.sync.dma_start(
                        wm[:, 0:NBLK_DM * 128]
                        .rearrange("p (k c) -> p k c", k=NBLK_DM),
                        w_in_r[:, :, m * 128:(m + 1) * 128])
                    if conv_on_pe and m < NBLK_DF:
                        nc.sync.dma_start(
                            wm[:, NBLK_DM * 128:],
                            w_cdiag[:, m * K4 * 128:(m + 1) * K4 * 128])
                    ps = apsum.tile([128, TC], F32, tag="inproj", name="inproj")
                    for k in range(NBLK_DM):
                        nc.tensor.matmul(ps[:], wm[:, k * 128:(k + 1) * 128],
                                         x_sb[k][:], start=(k == 0),
                                         stop=(k == NBLK_DM - 1))
                    if m < NBLK_DF:
                        xi = apool.tile([128, 3 + TC], BF16, tag="xi", name="xi")
                        nc.gpsimd.tensor_copy(xi[:, 0:3], halo[m][:])
                        nc.scalar.activation(xi[:, 3:3 + TC], ps[:], AF.Copy)
                        nc.scalar.activation(halo[m][:], ps[:, TC - 3:TC],
                                             AF.Copy)
                        xc_t = xcpool.tile([128, TC], BF16, tag=f"xco{m}",
                                           name=f"xco{m}")
                        if conv_on_pe:
                            psc = apsum.tile([128, TC], F32, tag="inproj",
                                             name="convps")
                            for kk in range(K4):
                                nc.tensor.matmul(
                                    psc[:],
                                    wm[:, (NBLK_DM + kk) * 128:
                                       (NBLK_DM + kk + 1) * 128],
                                    xi[:, kk:kk + TC],
                                    start=(kk == 0), stop=(kk == K4 - 1))
                            nc.scalar.activation(xc_t[:], psc[:], AF.Silu,
                                                 bias=conv_b_sb[:, m:m + 1])
                        else:
                            acc = apool.tile([128, TC], BF16, tag="convacc",
                                             name="convacc")
                            tmp = apool.tile([128, TC], BF16, tag="convtmp",
                                             name="convtmp")
                            nc.vector.tensor_scalar(
                                acc[:], xi[:, 0:TC],
                                conv_w_sb[:, m * K4:m * K4 + 1], None, ALU.mult)
                            for kk in range(1, K4):
                                nc.vector.tensor_scalar(
                                    tmp[:], xi[:, kk:kk + TC],
                                    conv_w_sb[:, m * K4 + kk:m * K4 + kk + 1],
                                    None, ALU.mult)
                                nc.vector.tensor_tensor(acc[:], acc[:], tmp[:],
                                                        ALU.add)
                            nc.scalar.activation(xc_t[:], acc[:], AF.Silu,
                                                 bias=conv_b_sb[:, m:m + 1])
                        if m < NBLK_DH:
                            nc.sync.dma_start(
                                xc_spill[m * 128:(m + 1) * 128, t0:t0 + TC],
                                xc_t[:])
                        xc_chunk.append(xc_t)
                    else:
                        zb = m - NBLK_DF
                        zt = apool.tile([128, TC], BF16, tag="zt", name="zt")
                        nc.scalar.activation(zt[:], ps[:], AF.Silu)
                        nc.sync.dma_start(
                            z_spill[zb * 128:(zb + 1) * 128, t0:t0 + TC], zt[:])

                # xproj
                ps96 = apsum1.tile([R + 2 * N, TC], F32, tag="xpdt", name="xproj")
                for k in range(NBLK_DF):
                    nc.tensor.matmul(ps96[:], w_xp_sb[k][:], xc_chunk[k][:],
                                     start=(k == 0), stop=(k == NBLK_DF - 1))
                xdbl = apool.tile([R + 2 * N, TC], BF16, tag="xdbl", name="xdbl")
                nc.scalar.activation(xdbl[:], ps96[:], AF.Copy)
                nc.sync.dma_start(bc_spill[:, t0:t0 + TC], xdbl[R:R + 2 * N, :])
                # dt proj + softplus; Exp x4 then Ln x4 batched so the
                # activation-table pass emits one load per run, not per op.
                spes = []
                for mb in range(NBLK_DH):
                    psd = apsum1.tile([128, TC], F32, tag="xpdt", name="dtproj")
                    nc.tensor.matmul(psd[:], w_dt_sb[:, mb * 128:(mb + 1) * 128],
                                     xdbl[0:R, :], start=True, stop=True)
                    spe = dtpool.tile([128, TC], BF16, tag=f"spe{mb % 4}",
                                      name="spe")
                    nc.scalar.activation(spe[:], psd[:], AF.Exp,
                                         bias=dt_b_sb[:, mb:mb + 1])
                    spes.append(spe)
                    if mb % 4 == 3:
                        for j, sp in enumerate(spes):
                            mbj = mb - 3 + j
                            dtt = dtpool.tile([128, TC], BF16, tag=f"dtt{j % 2}",
                                              name="dtt")
                            nc.scalar.activation(dtt[:], sp[:], AF.Ln, bias=1.0)
                            nc.sync.dma_start(
                                dt_spill[mbj * 128:(mbj + 1) * 128, t0:t0 + TC],
                                dtt[:])
                            bst = dtpool.tile([128, TC], BF16, tag=f"bst{j % 2}",
                                              name="bst")
                            nc.gpsimd.tensor_tensor(bst[:], dtt[:],
                                                    xc_chunk[mbj][:], ALU.mult)
                            nc.sync.dma_start(
                                bsc_spill[mbj * 128:(mbj + 1) * 128,
                                          t0:t0 + TC], bst[:])
                        spes = []
                # after chunks 1..3: inject one full phase-B h0 pair (14 steps)
                if c >= 1:
                    for _ in range(14):
                        nxt = next(stepsB0, None)
                        if nxt is not None:
                            nxt()

        # ====== drain B h0; load w_out; B h1 interleaved with C h0 ======
        for nxt in stepsB0:
            nxt()
        with tc.tile_pool(name="phaseC_w", bufs=1) as wpc:
            w_out_sb = []
            for k in range(NBLK_DH):
                t = wpc.tile([128, DM], BF16, tag=f"w_out{k}", name=f"w_out{k}")
                nc.sync.dma_start(t[:], w_out[k * 128:(k + 1) * 128, :])
                w_out_sb.append(t)
            stepsC0 = iter(make_phaseC_steps(0, w_out_sb))
            stepsB1 = make_phaseB_steps(1)
            for i, stp in enumerate(stepsB1):
                stp()
                if i % 14 >= 10:
                    nxt = next(stepsC0, None)
                    if nxt is not None:
                        nxt()
            for nxt in stepsC0:
                nxt()
            for stp in make_phaseC_steps(1, w_out_sb):
                stp()


def _prep_inputs(inputs):
    """Build the 8 per-core input maps from full inputs (numpy fp32)."""
    bf = ml_dtypes.bfloat16
    x = np.asarray(inputs["x"], np.float32)
    maps = []
    for core in range(8):
        dire, bat, half = core // 4, (core // 2) % 2, core % 2
        p = "fwd" if dire == 0 else "bwd"
        in_W = np.asarray(inputs[p + "_in_W"], np.float32)
        conv_w = np.asarray(inputs[p + "_conv_w"], np.float32)
        conv_b = np.asarray(inputs[p + "_conv_b"], np.float32)
        xproj_W = np.asarray(inputs[p + "_xproj_W"], np.float32)
        dt_W = np.asarray(inputs[p + "_dt_W"], np.float32)
        dt_b = np.asarray(inputs[p + "_dt_b"], np.float32)
        A_log = np.asarray(inputs[p + "_A_log"], np.float32)
        Dvec = np.asarray(inputs[p + "_D"], np.float32)
        out_W = np.asarray(inputs[p + "_out_W"], np.float32)
        proj_W = np.asarray(inputs["proj_W"], np.float32)

        # the kernel generates dA = exp(-n*dt); verify A has that structure
        A = -np.exp(A_log)
        assert np.allclose(A, -np.arange(1, N + 1, dtype=np.float32)[None, :]
                           .repeat(DI, 0), atol=1e-4), "unexpected A structure"

        own = slice(half * DH, (half + 1) * DH)
        xb = x[bat]
        if dire == 1:
            xb = xb[::-1]
        perm = np.concatenate([np.arange(half * DH, (half + 1) * DH),
                               np.arange((1 - half) * DH, (2 - half) * DH)])
        w_in_cat = np.concatenate(
            [in_W[perm], in_W[DI + half * DH:DI + (half + 1) * DH]], 0)
        W_eff = proj_W[:, dire * DM:(dire + 1) * DM] @ out_W   # (DM, DI)
        D_own = Dvec[own]
        w_diag = np.zeros((128, NBLK_DH * 128), np.float32)
        for b in range(NBLK_DH):
            w_diag[:, b * 128:(b + 1) * 128] = np.diag(D_own[b * 128:(b + 1) * 128])
        cw = conv_w[perm]    # (DI, 4)
        w_cdiag = np.zeros((128, NBLK_DF * K4 * 128), np.float32)
        for m in range(NBLK_DF):
            for kk in range(K4):
                j = (m * K4 + kk) * 128
                w_cdiag[:, j:j + 128] = np.diag(cw[m * 128:(m + 1) * 128, kk])
        m = {
            "xT": np.ascontiguousarray(xb.T).astype(bf),
            "w_in": np.ascontiguousarray(w_in_cat.T).astype(bf),
            "w_xp": np.ascontiguousarray(xproj_W[:, perm].T).astype(bf),
            "w_dt": np.ascontiguousarray(dt_W[own].T).astype(bf),
            "w_out": np.ascontiguousarray(W_eff[:, own].T).astype(bf),
            "w_cdiag": np.ascontiguousarray(w_cdiag).astype(bf),
            "conv_w": np.ascontiguousarray(cw),
            "conv_b": np.ascontiguousarray(conv_b[perm][:, None]),
            "dt_b": np.ascontiguousarray(dt_b[own][:, None]),
            "eye": np.eye(128, dtype=np.float32).astype(bf),
            "w_diag": np.ascontiguousarray(w_diag).astype(bf),
        }
        maps.append(m)
    return maps


def _unshard(results, inputs):
    parts = [r["out"].astype(np.float32) for r in results]
    proj_b = np.asarray(inputs["proj_b"], np.float32)
    out = np.empty((B, L, DM), np.float32)
    for bat in range(2):
        fwd = parts[0 * 4 + bat * 2 + 0] + parts[0 * 4 + bat * 2 + 1]
        bwd = parts[1 * 4 + bat * 2 + 0] + parts[1 * 4 + bat * 2 + 1]
        out[bat] = (fwd + bwd[:, ::-1]).T + proj_b[None, :]
    return out


def kernel(**inputs):
    if "nc" not in _CACHED:
        _CACHED["nc"] = _build_module()
    nc = _CACHED["nc"]
    maps = _prep_inputs(inputs)
    res = bass_utils.run_bass_kernel_spmd(nc, maps, core_ids=list(range(8)))
    return _unshard(res.results, inputs)
